# revision 3
# baseline (speedup 1.0000x reference)
"""Trainium2 Bass kernel for e3nn-style GNN message passing + segment-sum (v4).

v4 over v3: s0 folded into a second one-hot (oh_s0) so msg holds
[mix0*s | v*mix3 | dot*mix1 | (mix2*s)xv0] and the scatter stays 2 FWL
matmuls (win0 rhs=oh_s0, win1 rhs=oh; host un-permutes rows); MLP silus
packed [128, 256] via col-tiled matmuls; single rotating hp PSUM tag;
wf flushes off ScalarE.
"""

import math
import os

import numpy as np

P = 128          # edges per chunk == SBUF partitions
S = 16           # max node-span of a chunk (one-hot width)
WIN = 512        # nodes per PSUM window (one f32 bank)
C32 = 32         # irrep multiplicity
NCORES = 8
G = 16           # chunks per DVE fusion group
G1 = 4           # chunks per MLP matmul subgroup (512 cols)

_CACHE = {}
_LAST_RUN = [None, None]


def last_run():
    return _LAST_RUN[0], _LAST_RUN[1]


# ----------------------------------------------------------------- host prep

def _build_schedule(receivers, n_nodes, ncores):
    E = receivers.shape[0]
    npc = n_nodes // ncores
    assert n_nodes % ncores == 0
    nw = math.ceil(npc / WIN)
    order = np.argsort(receivers, kind="stable")
    rs = receivers[order]

    win_edges = {}
    K_ws = np.zeros(nw, dtype=np.int64)
    for c in range(ncores):
        for w in range(nw):
            lo = c * npc + w * WIN
            hi = c * npc + min((w + 1) * WIN, npc)
            a, b = np.searchsorted(rs, [lo, hi])
            win_edges[(c, w)] = (a, b, lo)
            K_ws[w] = max(K_ws[w], math.ceil(max(b - a, 1) / P))

    C = int(K_ws.sum())
    per_core = []
    for c in range(ncores):
        rel_t = np.full((P, C), -1.0, np.float32)
        off_t = np.zeros((1, C), np.int32)
        ids_km = np.zeros((C, P), np.int64)
        valid = np.zeros((C, P), bool)
        c0 = 0
        for w in range(nw):
            K = int(K_ws[w])
            a, b, base = win_edges[(c, w)]
            nreal = b - a
            tot = K * P
            ids = np.zeros(tot, np.int64)
            ids[:nreal] = order[a:b]
            nr = np.full(tot, -(10 ** 6), np.int64)
            nr[:nreal] = rs[a:b] - base
            ids_k = ids.reshape(K, P)
            nr_k = nr.reshape(K, P)
            off = nr_k[:, 0].copy()
            off[off < 0] = 0
            # clamp so off+S never crosses the WIN-column PSUM bank; edges
            # stay in range because node < WIN implies node - (WIN-S) < S
            np.minimum(off, WIN - S, out=off)
            rel = nr_k - off[:, None]
            bad = nr_k < 0
            rel[bad] = -1
            if rel.max(initial=0) >= S:
                raise AssertionError("chunk node-span exceeded S")
            rel_t[:, c0:c0 + K] = rel.T.astype(np.float32)
            off_t[0, c0:c0 + K] = off.astype(np.int32)
            ids_km[c0:c0 + K] = ids_k
            valid[c0:c0 + K] = ~bad
            c0 += K
        per_core.append((rel_t, off_t, ids_km, valid))
    return [int(k) for k in K_ws], per_core


def _pack_core(ef, ea, rel_t, ids_km, valid, bf16):
    """ef_pack [P, C*128], sT_pack [32, C*128],
    attrs [P, C*8] with slot (s0, rel, v0x, v0y, v0z, 0, 0, 0)."""
    Ck = ids_km.shape[0]
    rows = ef[ids_km]                          # [C, P, 128]
    rows[~valid] = 0.0
    at = np.zeros((Ck, P, 8), np.float32)
    at[:, :, 0:1] = ea[ids_km][:, :, 0:1]
    at[:, :, 2:5] = ea[ids_km][:, :, 1:4]
    at[~valid] = 0.0
    at[:, :, 1] = rel_t.T
    ef_pack = rows.transpose(1, 0, 2).reshape(P, Ck * P)
    sT_pack = rows[:, :, :C32].transpose(2, 0, 1).reshape(C32, Ck * P)
    attrs = at.transpose(1, 0, 2).reshape(P, Ck * 8)
    ef_pack = np.ascontiguousarray(ef_pack.astype(bf16))
    sT_pack = np.ascontiguousarray(sT_pack.astype(bf16))
    attrs = np.ascontiguousarray(attrs.astype(bf16))
    return ef_pack, sT_pack, attrs


# ------------------------------------------------------------- device build

def _build_program(K_ws, ncores, repeat=1):
    import concourse.bass as bass
    import concourse.bacc as bacc
    import concourse.mybir as mybir
    import concourse.tile as tile

    F32 = mybir.dt.float32
    BF16 = mybir.dt.bfloat16
    I32 = mybir.dt.int32
    C = sum(K_ws)
    nw = len(K_ws)

    nc = bacc.Bacc("TRN2", target_bir_lowering=False, debug=False,
                   num_devices=ncores)

    ef_d = nc.dram_tensor("efp", [P, C * P], BF16, kind="ExternalInput")
    sT_d = nc.dram_tensor("stp", [C32, C * P], BF16, kind="ExternalInput")
    attrs_d = nc.dram_tensor("attrs", [P, 8 * C], BF16, kind="ExternalInput")
    offt_d = nc.dram_tensor("offt", [1, C], I32, kind="ExternalInput")
    w1_d = nc.dram_tensor("w1", [C32, 2 * C32], BF16, kind="ExternalInput")
    w2_d = nc.dram_tensor("w2", [2 * C32, 2 * C32], BF16,
                          kind="ExternalInput")
    w3_d = nc.dram_tensor("w3", [2 * C32, 4 * C32], BF16,
                          kind="ExternalInput")
    iota_d = nc.dram_tensor("iota", [P, S], BF16, kind="ExternalInput")
    out_d = nc.dram_tensor("outfm", [2 * P, nw * WIN], F32,
                           kind="ExternalOutput")

    with tile.TileContext(nc) as tc:
        with tc.tile_pool(name="const", bufs=1) as cpool, \
             tc.tile_pool(name="sb", bufs=4) as sb, \
             tc.tile_pool(name="wtab", bufs=2) as wtab, \
             tc.tile_pool(name="wflush", bufs=2) as wf, \
             tc.tile_pool(name="psH", bufs=2, space="PSUM") as psH, \
             tc.tile_pool(name="psM", bufs=2, space="PSUM") as psM, \
             tc.tile_pool(name="pswin", bufs=2, space="PSUM") as pswin:

            w1 = cpool.tile([C32, 2 * C32], BF16)
            w2 = cpool.tile([2 * C32, 2 * C32], BF16)
            w3 = cpool.tile([2 * C32, 4 * C32], BF16)
            # copies on partitions 64:128 for col-tiled (upper-half) matmuls
            w2b = cpool.tile([P, 2 * C32], BF16)
            w3b = cpool.tile([P, 4 * C32], BF16)
            iota = cpool.tile([P, S], BF16)
            nc.sync.dma_start(w1[:], w1_d[:])
            nc.sync.dma_start(w2[:], w2_d[:])
            nc.sync.dma_start(w3[:], w3_d[:])
            nc.sync.dma_start(w2b[2 * C32:4 * C32, :], w2_d[:])
            nc.sync.dma_start(w3b[2 * C32:4 * C32, :], w3_d[:])
            nc.sync.dma_start(iota[:], iota_d[:])
            zcol = cpool.tile([1, P], BF16)
            zrow = cpool.tile([1, WIN], BF16)
            nc.vector.memset(zcol[:], 0.0)
            nc.vector.memset(zrow[:], 0.0)

            import contextlib
            rep_ctx = (tc.For_i(0, repeat, 1) if repeat > 1
                       else contextlib.nullcontext())
            with rep_ctx:
                _emit_body(nc, tc, bass, mybir, tile, K_ws, sb, wtab, wf,
                           psH, psM, pswin, w1, w2, w2b, w3, w3b, iota,
                           zcol, zrow, ef_d, sT_d, attrs_d, offt_d, out_d)

    nc.compile()
    return nc


def _emit_body(nc, tc, bass, mybir, tile, K_ws, sb, wtab, wf, psH, psM,
               pswin, w1, w2, w2b, w3, w3b, iota, zcol, zrow, ef_d, sT_d,
               attrs_d, offt_d, out_d):
    F32 = mybir.dt.float32
    BF16 = mybir.dt.bfloat16
    I32 = mybir.dt.int32
    AF = mybir.ActivationFunctionType
    ALU = mybir.AluOpType
    nw = len(K_ws)
    Kmax = max(K_ws)
    H = 2 * C32
    cidx = 0
    for w in range(nw):
        K = K_ws[w]
        # one buffer per window: sidesteps a missed WAR edge between the
        # fused-AP readers of window w and window w+2's table DMA
        attrs_w = wtab.tile([P, 8 * Kmax], BF16, tag="attrs", bufs=nw)
        offt_w = wtab.tile([1, Kmax], I32, tag="offt", bufs=nw)
        nc.scalar.dma_start(attrs_w[:, 0:8 * K],
                            attrs_d[:, 8 * cidx:8 * (cidx + K)])
        nc.scalar.dma_start(offt_w[:, 0:K], offt_d[:, cidx:cidx + K])

        win0 = pswin.tile([P, WIN], F32, tag="win0", space="PSUM")
        win1 = pswin.tile([P, WIN], F32, tag="win1", space="PSUM")
        nc.tensor.matmul(win0[:], zcol[:], zrow[:], start=True, stop=False)
        nc.tensor.matmul(win1[:], zcol[:], zrow[:], start=True, stop=False)

        for g0 in range(0, K, G):
            Gc = min(G, K - g0)
            col0 = (cidx + g0) * P
            ef_g = sb.tile([P, G * P], BF16, tag="ef")
            nc.sync.dma_start(ef_g[:, 0:Gc * P],
                              ef_d[:, col0:col0 + Gc * P])
            sT_g = sb.tile([C32, G * P], BF16, tag="sT")
            nc.sync.dma_start(sT_g[:, 0:Gc * P],
                              sT_d[:, col0:col0 + Gc * P])
            mix_sb = sb.tile([P, G * P], BF16, tag="mix")

            # ---- MLP (batched): mix = silu(silu(s@W1)@W2)@W3, edge-major
            for h0 in range(0, Gc, G1):
                Hc = min(G1, Gc - h0)
                hw = Hc * P
                base = h0 * P
                h1p = psH.tile([H, G1 * P], F32, tag="hp", space="PSUM")
                nc.tensor.matmul(h1p[:, 0:hw], w1[:],
                                 sT_g[:, base:base + hw],
                                 start=True, stop=True)
                h1s = sb.tile([H, G1 * P], BF16, tag="h1s")
                nc.scalar.activation(h1s[:, 0:hw], h1p[:, 0:hw], AF.Silu)
                h2p = psH.tile([H, G1 * P], F32, tag="hp", space="PSUM")
                nc.tensor.matmul(h2p[:, 0:hw], w2[:], h1s[:, 0:hw],
                                 start=True, stop=True)
                h2s = sb.tile([H, G1 * P], BF16, tag="h2s")
                nc.scalar.activation(h2s[:, 0:hw], h2p[:, 0:hw], AF.Silu)
                mixp = psM.tile([P, G1 * P], F32, tag="mixp", space="PSUM")
                for t in range(Hc):
                    nc.tensor.matmul(mixp[:, t * P:(t + 1) * P],
                                     h2s[:, t * P:(t + 1) * P], w3[:],
                                     start=True, stop=True)
                nc.scalar.activation(mix_sb[:, base:base + hw],
                                     mixp[:, 0:hw], AF.Copy)

            # ---- tensor-product messages, fused across the group.
            # msg cols: [mix0*s (32) | v*mix3 (96) | dot*mix1 (32) |
            #            (mix2*s) x v0 (96)] — win0 scatters cols 0:128
            # with rhs=oh*s0, win1 cols 128:256 with rhs=oh; the host
            # un-permutes output rows.
            a8 = attrs_w[:, 8 * g0:8 * (g0 + Gc)] \
                .rearrange("p (g a) -> p g a", a=8)
            s0 = a8[:, :, 0:1]                                # [P,Gc,1]
            rr = a8[:, :, 1:2]                                # [P,Gc,1]
            v0 = a8[:, :, 2:5].unsqueeze(2)                   # [P,Gc,1,3]
            efv = ef_g[:, 0:Gc * P].rearrange("p (g f) -> p g f", f=P)
            sA = efv[:, :, 0:C32]
            vA = efv[:, :, C32:4 * C32] \
                .rearrange("p g (c d) -> p g c d", d=3)       # [P,Gc,32,3]
            mixv = mix_sb[:, 0:Gc * P].rearrange("p (g f) -> p g f", f=P)
            mix0 = mixv[:, :, 0:C32]
            mix1 = mixv[:, :, C32:2 * C32]
            mix2 = mixv[:, :, 2 * C32:3 * C32]
            mix3 = mixv[:, :, 3 * C32:4 * C32]

            msg = sb.tile([P, G * 2 * P], BF16, tag="msg")
            msgv = msg[:, 0:Gc * 2 * P].rearrange("p (g f) -> p g f",
                                                  f=2 * P)
            tmpa = sb.tile([P, G * C32], BF16, tag="tmpa")
            tmpd = sb.tile([P, G * C32], BF16, tag="tmpd")
            tmpd2 = sb.tile([P, G * C32], BF16, tag="tmpd2")
            tmpu = sb.tile([P, G * 3 * C32], BF16, tag="tmpu")
            oh = sb.tile([P, G * S], BF16, tag="oh")
            ohs = sb.tile([P, G * S], BF16, tag="ohs")

            tav = tmpa[:, 0:Gc * C32].rearrange("p (g c) -> p g c", c=C32)
            tdv = tmpd[:, 0:Gc * C32].rearrange("p (g c) -> p g c", c=C32)
            td2v = tmpd2[:, 0:Gc * C32].rearrange("p (g c) -> p g c", c=C32)
            tuv = tmpu[:, 0:Gc * 3 * C32] \
                .rearrange("p (g c d) -> p g c d", c=C32, d=3)
            ohv = oh[:, 0:Gc * S].rearrange("p (g j) -> p g j", j=S)
            ohsv = ohs[:, 0:Gc * S].rearrange("p (g j) -> p g j", j=S)

            v0_cd = v0.to_broadcast([P, Gc, C32, 3])

            # msg[0:32] = mix0 * s ; tmpa = mix2 * s   (DVE, 2x bf16)
            nc.vector.tensor_tensor(msgv[:, :, 0:C32], mix0, sA,
                                    op=ALU.mult)
            nc.vector.tensor_tensor(tav, mix2, sA, op=ALU.mult)
            # u = v * v0 ; dot = u0+u1+u2 ; msg[128:160] = dot * mix1
            nc.vector.tensor_tensor(tuv, vA, v0_cd, op=ALU.mult)
            nc.vector.tensor_tensor(tdv.unsqueeze(3), tuv[:, :, :, 0:1],
                                    tuv[:, :, :, 1:2], op=ALU.add)
            nc.vector.tensor_tensor(td2v.unsqueeze(3), tdv.unsqueeze(3),
                                    tuv[:, :, :, 2:3], op=ALU.add)
            nc.vector.tensor_tensor(msgv[:, :, 4 * C32:5 * C32], td2v,
                                    mix1, op=ALU.mult)
            # msg[32:128] = v * mix3           (GPSIMD)
            nc.gpsimd.tensor_tensor(
                msgv[:, :, C32:4 * C32]
                .rearrange("p g (c d) -> p g c d", d=3),
                vA, mix3.unsqueeze(3).to_broadcast([P, Gc, C32, 3]),
                op=ALU.mult)
            # msg[160:256] = tmpa[c] * v0[d]   (GPSIMD)
            nc.gpsimd.tensor_tensor(
                msgv[:, :, 5 * C32:8 * C32]
                .rearrange("p g (c d) -> p g c d", d=3),
                tav.unsqueeze(3).to_broadcast([P, Gc, C32, 3]),
                v0_cd, op=ALU.mult)
            # one-hot(rel) and s0-weighted one-hot   (DVE)
            nc.vector.tensor_tensor(
                ohv, iota[:].unsqueeze(1).to_broadcast([P, Gc, S]),
                rr.to_broadcast([P, Gc, S]), op=ALU.is_equal)
            nc.vector.tensor_tensor(ohsv, ohv,
                                    s0.to_broadcast([P, Gc, S]),
                                    op=ALU.mult)

            # ---- accumulate into window PSUM at dynamic column offset
            for t in range(Gc):
                k = g0 + t
                reg = nc.alloc_register(mybir.EngineType.PE)
                nc.tensor.reg_load(reg, offt_w[0:1, k:k + 1])
                off = nc.snap(reg, donate=True, min_val=0,
                              max_val=WIN - S)
                nc.tensor.matmul(win0[:, bass.ds(off, S)],
                                 msg[:, t * 2 * P:t * 2 * P + P],
                                 ohs[:, t * S:(t + 1) * S],
                                 start=False, stop=False,
                                 skip_group_check=True)
                nc.tensor.matmul(win1[:, bass.ds(off, S)],
                                 msg[:, t * 2 * P + P:(t + 1) * 2 * P],
                                 oh[:, t * S:(t + 1) * S],
                                 start=False, stop=False,
                                 skip_group_check=True)

        # close accumulation groups, flush window
        nc.tensor.matmul(win0[:], zcol[:], zrow[:], start=False, stop=True)
        nc.tensor.matmul(win1[:], zcol[:], zrow[:], start=False, stop=True)
        wf0 = wf.tile([P, WIN], F32, tag="wf0")
        wf1 = wf.tile([P, WIN], F32, tag="wf1")
        nc.vector.tensor_copy(wf0[:], win0[:])
        nc.vector.tensor_copy(wf1[:], win1[:])
        nc.sync.dma_start(out_d[0:P, w * WIN:(w + 1) * WIN], wf0[:])
        nc.sync.dma_start(out_d[P:2 * P, w * WIN:(w + 1) * WIN], wf1[:])
        cidx += K


# ------------------------------------------------------------------- kernel

def kernel(edge_feats, edge_attrs, receivers, n_nodes, W1, W2, W3):
    from concourse.bass_utils import run_bass_kernel_spmd
    import concourse.mybir as mybir

    bf16 = mybir.dt.np(mybir.dt.bfloat16)
    ef = np.ascontiguousarray(np.asarray(edge_feats, dtype=np.float32))
    ea = np.ascontiguousarray(np.asarray(edge_attrs, dtype=np.float32))
    rc = np.asarray(receivers).astype(np.int64)
    n = int(n_nodes)
    W1 = np.asarray(W1, dtype=np.float32)
    W2 = np.asarray(W2, dtype=np.float32)
    W3 = np.asarray(W3, dtype=np.float32)
    npc = n // NCORES
    nw = math.ceil(npc / WIN)

    K_ws, per_core = _build_schedule(rc, n, NCORES)

    w1s = (W1 / math.sqrt(W1.shape[0])).astype(bf16)
    w2s = (W2 / math.sqrt(W2.shape[0])).astype(bf16)
    w3s = (W3 / math.sqrt(W3.shape[0])).astype(np.float32)
    colscale = np.full(4 * C32, 1.0 / math.sqrt(20.0), np.float32)
    colscale[C32:2 * C32] /= math.sqrt(3.0)
    w3s = (w3s * colscale[None, :]).astype(bf16)
    iota = np.tile(np.arange(S, dtype=np.float32), (P, 1)).astype(bf16)

    repeat = int(os.environ.get("KERNEL_REPEAT", "1"))
    key = (tuple(K_ws), repeat)
    if key not in _CACHE:
        _CACHE[key] = _build_program(K_ws, NCORES, repeat=repeat)
    nc = _CACHE[key]

    in_maps = []
    for c in range(NCORES):
        rel_t, off_t, ids_km, valid = per_core[c]
        ef_pack, sT_pack, attrs = _pack_core(ef, ea, rel_t, ids_km, valid,
                                             bf16)
        in_maps.append({
            "efp": ef_pack,
            "stp": sT_pack,
            "attrs": attrs,
            "offt": off_t,
            "w1": w1s, "w2": w2s, "w3": w3s,
            "iota": iota,
        })

    _LAST_RUN[0], _LAST_RUN[1] = nc, in_maps
    res = run_bass_kernel_spmd(nc, in_maps, core_ids=list(range(NCORES)))
    if res.exec_time_ns is not None:
        print(f"HW exec time: {res.exec_time_ns} ns")

    # device row order: win0=[s1, v2], win1=[s2, v1] -> [s1, s2, v1, v2]
    perm = np.concatenate([np.arange(0, 32), np.arange(128, 160),
                           np.arange(160, 256), np.arange(32, 128)])
    out = np.empty((n, 2 * P), np.float32)
    for c in range(NCORES):
        fm = res.results[c]["outfm"][perm]
        for w in range(nw):
            lo = w * WIN
            ln = min(WIN, npc - lo)
            rows = slice(c * npc + lo, c * npc + lo + ln)
            out[rows, :] = fm[:, lo:lo + ln].T
    return out
